# revision 1
# baseline (speedup 1.0000x reference)
"""Trainium2 Bass kernel for nn_DotMatrix.

Math: for each (b, ell, t) the reference computes a complex pairwise dot
matrix O[i,j] = sum_m z[i,m] * w[j,m] where z = rep[b,:,t,:,:] as complex
and w the sign-flipped conjugation partner.  As a real matmul:

  lhsT[k, i]   k = (c,m) stacked: [Zr.T; Zi.T]                 [2m, 256]
  rhs[k, 2j+c'] c'=0: [FZr; -FZi], c'=1: [FZi; FZr]            [2m, 512]
  out = lhsT.T @ rhs  -> [256 i, 512 (j,c)]

with FZr[m',j] = s[m'] * Zr[j, M-1-m'], s[m'] = (-1)^(ell+m').

Precision trick: fp32 matmuls run at 4 cycles/column on the PE, but the
contraction dim here is tiny (2m <= 14), so we decompose each operand
into three bf16 parts (hi/mid/lo, 24 mantissa bits total) and stack the
six significant cross terms along the dead K dimension:

  L = [Ah; Am; Al; Ah; Am; Ah]   R = [Bh; Bh; Bh; Bm; Bm; Bl]

One bf16 matmul (K = 6*2m <= 84) then equals the fp32 product to
~2^-24, at 1 cycle/column — 4x faster than the fp32 path and with fast
(FWL) weight loads.

Symmetry trick: the pairwise matrix is symmetric in (i,j) for both the
real and imaginary components (O[i,j] = O[j,i]), so each channel only
computes 32-row i-blocks against j >= 32*bi — 56.25% of the matrix —
and the host mirrors the lower block-triangle for free.

Sharding: 8 cores = 2 batches x 4 tau-quarters.  Each core owns 32
channels ch = ell*8 + s (t = tq*8 + s).  Four channels (a quad) share
each matmul's 128 PSUM partitions via column tiling (tile_position),
the PSUM is evacuated by alternating ScalarE/VectorE copies into a
[128, 2304] staging tile, and each quad leaves as one contiguous
1.18MB HWDGE store.  Inputs are partition-packed into one full-height
[128, 12288] tensor (ell3+ell0 rows 0:84/96:108, ell2+ell1 rows
0:60/64:100 — matmul base partitions are restricted to 0/32/64/96)
loaded as three large full-width chunks so input DMA uses all 16
SBUF ports with minimal descriptor-emission overhead, and a short
dependency-free dummy-matmul chain pre-warms the PE clock gate.
Host reassembles the full [2,256,256,128,2] output.
"""

import numpy as np
import ml_dtypes

import concourse.bass as bass
import concourse.bacc as bacc
import concourse.mybir as mybir
from concourse.bass_utils import run_bass_kernel_spmd
from concourse.tile import TileContext

B, N, TAU, NELL = 2, 256, 32, 4
NCORES = 8
NCH = 32          # channels per core (4 ell * 8 slots)
F32 = mybir.dt.float32
BF16 = mybir.dt.bfloat16
BFNP = ml_dtypes.bfloat16
KS = [6 * 2 * (2 * ell + 1) for ell in range(NELL)]   # 12, 36, 60, 84
BIW = [512 - 64 * bi for bi in range(8)]              # cols per 32-row i-block
BIO = [0, 512, 960, 1344, 1664, 1920, 2112, 2240]     # ot offsets per i-block
OTW = 2304                                            # sum(BIW)

_NC_CACHE = {}


def _build_bass():
    nc = bacc.Bacc()
    # Single packed input: cols [0:3072) = A slots 0-3 (critical first
    # chunk), [3072:6144) = A slots 4-7, [6144:12288) = B; rows 0:84 =
    # ell3, 96:108 = ell0 (A-cols), 0:60 = ell2, 64:100 = ell1 (B-cols).
    inp_d = nc.declare_dram_parameter("inp", [128, 12288], BF16, isOutput=False)
    # The pairwise matrix is symmetric in (i,j), so each channel only
    # computes i-blocks of 32 against j >= 32*bi (block upper triangle,
    # 56.25% of the full matrix); the host mirrors the rest.  Four channels
    # (a quad) share each matmul's 128 PSUM partitions via column tiling.
    # Layout: [quad, psum_row, (bi-block columns)] — contiguous per quad.
    out = nc.declare_dram_parameter("out", [NCH // 4, 128, OTW], F32, isOutput=True)

    with TileContext(nc) as tc:
        with (
            tc.tile_pool(name="lin", bufs=1) as lin_pool,
            tc.tile_pool(name="rin", bufs=1) as rin_pool,
            tc.tile_pool(name="ps", bufs=8, space="PSUM") as ps_pool,
            tc.tile_pool(name="ot", bufs=5) as ot_pool,
        ):
            in_sb = lin_pool.tile([128, 12288], BF16, name="in_sb")
            # PE pre-warm: dependency-free dummy matmuls on scratch tiles keep
            # the PE busy from kernel start, so the HAM clock gate is already
            # released (2.4 GHz) when the first real matmuls arrive.
            warm_in = lin_pool.tile([128, 512], BF16, name="warm_in")
            warm_ps = ps_pool.tile([128, 512], F32, tag="ps", name="warm_ps")
            nc.vector.memset(warm_in[:], 0.0)
            for _ in range(10):
                nc.tensor.matmul(
                    warm_ps[:], warm_in[:, 0:128], warm_in[:, 0:512],
                    start=True, stop=True,
                )
            # ell -> (packed tensor idx, base partition)
            pack = {3: (0, 0), 0: (0, 96), 2: (1, 0), 1: (1, 64)}
            # Input loads ride the sync HWDGE ring ahead of the output
            # stores (strict FIFO, single queue keeps full engine attention);
            # three large full-width chunks minimize descriptor-emission
            # overhead while the first chunk unblocks the A quads early.
            nc.sync.dma_start(out=in_sb[:, 0:3072], in_=inp_d[:, 0:3072])
            nc.sync.dma_start(out=in_sb[:, 3072:6144], in_=inp_d[:, 3072:6144])
            nc.sync.dma_start(out=in_sb[:, 6144:12288], in_=inp_d[:, 6144:12288])
            n_copy = 0
            quad_order = [(0, 0), (3, 0), (0, 1), (3, 1), (2, 0), (1, 0), (2, 1), (1, 1)]
            for e, v in quad_order:
                K = KS[e]
                t, bp = pack[e]
                ot = ot_pool.tile([128, OTW], F32)
                for bi in range(8):     # i-block of 32 rows
                    W = BIW[bi]
                    ps = ps_pool.tile([128, 512], F32)
                    for c4 in range(4):  # channel within quad
                        sl = v * 4 + c4
                        lo = _lhs_off(t, sl)
                        ro = _rhs_off(t, sl)
                        nc.tensor.matmul(
                            ps[c4 * 32 : (c4 + 1) * 32, 0:W],
                            in_sb[
                                bp : bp + K,
                                lo + bi * 32 : lo + bi * 32 + 32,
                            ],
                            in_sb[
                                bp : bp + K, ro + 64 * bi : ro + 512
                            ],
                            start=True,
                            stop=True,
                            tile_position=(bp, c4 * 32),
                        )
                    dst = ot[:, BIO[bi] : BIO[bi] + W]
                    if n_copy % 2 == 0:
                        nc.scalar.copy(dst, ps[:, 0:W])
                    else:
                        nc.vector.tensor_copy(out=dst, in_=ps[:, 0:W])
                    n_copy += 1
                qidx = e * 2 + v
                # alternate stores across the two physical HWDGE rings so
                # SDMA engines can pull from the other ring at store
                # boundaries instead of idling
                eng = nc.sync if qidx % 2 == 0 else nc.scalar
                eng.dma_start(out=out[qidx], in_=ot[:])
    nc.compile()
    return nc


def _dec3(x):
    h = x.astype(BFNP)
    r = x - h.astype(np.float32)
    m_ = r.astype(BFNP)
    l = (r - m_.astype(np.float32)).astype(BFNP)
    return h, m_, l


_PACK = {3: (0, 0), 0: (0, 96), 2: (1, 0), 1: (1, 64)}


def _lhs_off(t, sl):
    # column of slot sl's lhsT block inside the packed input tile
    if t == 0:
        return sl * 256 if sl < 4 else 3072 + (sl - 4) * 256
    return 6144 + sl * 256


def _rhs_off(t, sl):
    if t == 0:
        return 1024 + sl * 512 if sl < 4 else 4096 + (sl - 4) * 512
    return 8192 + sl * 512


def _host_prep(reps, cid):
    """Build per-core bf16 K-stacked lhs/rhs tensors (partition-packed)."""
    b, tq = cid // 4, cid % 4
    im = {"inp": np.zeros((128, 12288), BFNP)}
    for ell in range(NELL):
        rep = reps[ell]
        m = 2 * ell + 1
        s_vec = ((-1.0) ** (ell + np.arange(m))).astype(np.float32)
        tp_, bp = _PACK[ell]
        INP = im["inp"]
        for sidx in range(8):
            t = tq * 8 + sidx
            Z = rep[b, :, t]                      # [256, m, 2]
            Zr, Zi = Z[..., 0], Z[..., 1]         # [256, m]
            lhsT = np.concatenate([Zr.T, Zi.T], axis=0)      # [2m, 256]
            FZr = s_vec[:, None] * Zr[:, ::-1].T             # [m, 256]
            FZi = s_vec[:, None] * Zi[:, ::-1].T
            R = np.empty((2 * m, 256, 2), np.float32)
            R[0:m, :, 0] = FZr
            R[m:, :, 0] = -FZi
            R[0:m, :, 1] = FZi
            R[m:, :, 1] = FZr
            rhs = R.reshape(2 * m, 512)
            Ah, Am, Al = _dec3(lhsT)
            Bh, Bm, Bl = _dec3(rhs)
            lo = _lhs_off(tp_, sidx)
            ro = _rhs_off(tp_, sidx)
            INP[bp : bp + KS[ell], lo : lo + 256] = np.concatenate(
                [Ah, Am, Al, Ah, Am, Ah], axis=0
            )
            INP[bp : bp + KS[ell], ro : ro + 512] = np.concatenate(
                [Bh, Bh, Bh, Bm, Bm, Bl], axis=0
            )
    return im


def _run(in_maps, **kw):
    if "nc" not in _NC_CACHE:
        _NC_CACHE["nc"] = _build_bass()
    return run_bass_kernel_spmd(_NC_CACHE["nc"], in_maps, list(range(NCORES)), **kw)


def kernel(rep0, rep1, rep2, rep3, _bass_kw=None):
    reps = [np.ascontiguousarray(np.asarray(r, dtype=np.float32)) for r in (rep0, rep1, rep2, rep3)]
    in_maps = [_host_prep(reps, cid) for cid in range(NCORES)]
    res = _run(in_maps, **(_bass_kw or {}))
    out = np.empty((B, N, N, NELL * TAU, 2), np.float32)
    for cid in range(NCORES):
        b, tq = cid // 4, cid % 4
        arr = res.results[cid]["out"]          # [8, 128, OTW]
        o = np.empty((NCH, 256, 256, 2), np.float32)
        for bi in range(8):
            nj = 256 - 32 * bi
            blk = arr[:, :, BIO[bi] : BIO[bi] + BIW[bi]].reshape(
                NCH // 4, 4, 32, nj, 2
            )
            # blk[quad, c4, i_in_block, j - 32*bi, c]; ch = 4*quad + c4
            for c4 in range(4):
                o[c4::4, 32 * bi : 32 * bi + 32, 32 * bi :, :] = blk[:, c4]
        for bi in range(1, 8):                  # mirror lower block triangle
            r = slice(32 * bi, 32 * bi + 32)
            o[:, r, : 32 * bi, :] = o[:, : 32 * bi, r, :].transpose(0, 2, 1, 3)
        for ell in range(NELL):
            lo = ell * TAU + tq * 8
            out[b, :, :, lo : lo + 8, :] = o[ell * 8 : (ell + 1) * 8].transpose(
                1, 2, 0, 3
            )
    kernel.last_result = res
    return out



# revision 2
# speedup vs baseline: 1.3371x; 1.3371x over previous
"""Trainium2 Bass kernel for nn_DotMatrix.

Math: for each (b, ell, t) the reference computes a complex pairwise dot
matrix O[i,j] = sum_m z[i,m] * w[j,m] where z = rep[b,:,t,:,:] as complex
and w the sign-flipped conjugation partner.  As a real matmul:

  lhsT[k, i]   k = (c,m) stacked: [Zr.T; Zi.T]                 [2m, 256]
  rhs[k, 2j+c'] c'=0: [FZr; -FZi], c'=1: [FZi; FZr]            [2m, 512]
  out = lhsT.T @ rhs  -> [256 i, 512 (j,c)]

with FZr[m',j] = s[m'] * Zr[j, M-1-m'], s[m'] = (-1)^(ell+m').

Precision: fp16 operands with fp32 PSUM accumulation give ~4e-4 relative
error on the final output (gate is 2e-2), so no multi-term decomposition
is needed — the contraction dim stays at K = 2m <= 14 and the input
tensors total just 393KB per core.  The output is likewise stored as
fp16 (~2e-4 elementwise), halving the store traffic that dominated the
f32 version's runtime.

Symmetry trick: the pairwise matrix is symmetric in (i,j) for both the
real and imaginary components (O[i,j] = O[j,i]), so each channel only
computes 32-row i-blocks against j >= 32*bi — 56.25% of the matrix —
and the host mirrors the lower block-triangle for free.

Sharding: 8 cores = 2 batches x 4 tau-quarters.  Each core owns 32
channels ch = ell*8 + s (t = tq*8 + s).  Four channels (a quad) share
each matmul's 128 PSUM partitions via column tiling (tile_position) —
the four streams run concurrently on disjoint 32-column PE tiles, so a
quad i-block costs one W-column pass.  Each ell lives in its own
32-partition row group (rows 32*ell + [0, 2m)), PSUM is evacuated by
alternating ScalarE/VectorE copies (f32 -> fp16) into a [128, 2304]
staging tile, and each quad leaves as one contiguous 590KB HWDGE store
alternating between the sync and scalar rings.  Host reassembles the
full [2,256,256,128,2] output.
"""

import numpy as np

import concourse.bass as bass
import concourse.bacc as bacc
import concourse.mybir as mybir
from concourse.bass_utils import run_bass_kernel_spmd
from concourse.tile import TileContext

B, N, TAU, NELL = 2, 256, 32, 4
NCORES = 8
NCH = 32          # channels per core (4 ell * 8 slots)
F32 = mybir.dt.float32
F16 = mybir.dt.float16
KS = [2 * (2 * ell + 1) for ell in range(NELL)]       # 2, 6, 10, 14
BIW = [512 - 64 * bi for bi in range(8)]              # cols per 32-row i-block
BIO = [0, 512, 960, 1344, 1664, 1920, 2112, 2240]     # ot offsets per i-block
OTW = 2304                                            # sum(BIW)
# quad qidx -> (ell, v); channels of a quad are slots 4v..4v+3 of that ell
QUAD_ORDER = [(0, 0), (0, 1), (3, 0), (3, 1), (2, 0), (2, 1), (1, 0), (1, 1)]
IN_COLS = 8 * 256 + 8 * 512                           # 6144: lhs slots then rhs slots

_NC_CACHE = {}


def _build_bass():
    nc = bacc.Bacc()
    # One input tensor per ell: [2m, 6144] fp16; cols [0:2048) hold the
    # eight 256-wide lhsT slot blocks, [2048:6144) the eight 512-wide rhs
    # slot blocks.  Each lands in SBUF row group 32*ell.
    inps = [
        nc.declare_dram_parameter(f"inp{e}", [KS[e], IN_COLS], F16, isOutput=False)
        for e in range(NELL)
    ]
    out = nc.declare_dram_parameter("out", [len(QUAD_ORDER), 128, OTW], F16, isOutput=True)

    with TileContext(nc) as tc:
        with (
            tc.tile_pool(name="lin", bufs=1) as lin_pool,
            tc.tile_pool(name="ps", bufs=8, space="PSUM") as ps_pool,
            tc.tile_pool(name="ot", bufs=5) as ot_pool,
        ):
            in_sb = lin_pool.tile([128, IN_COLS], F16, name="in_sb")
            # PE pre-warm: dependency-free dummy matmuls keep the PE busy
            # from kernel start so the HAM clock gate is already released
            # when the first real matmuls arrive.
            warm_in = lin_pool.tile([128, 512], F16, name="warm_in")
            warm_ps = ps_pool.tile([128, 512], F32, tag="ps", name="warm_ps")
            nc.vector.memset(warm_in[:], 0.0)
            for _ in range(4):
                nc.tensor.matmul(
                    warm_ps[:], warm_in[:, 0:128], warm_in[:, 0:512],
                    start=True, stop=True,
                )
            # Input loads ride the sync HWDGE ring, in the order quads
            # consume them (ell0 first).
            for e in (0, 3, 2, 1):
                nc.sync.dma_start(
                    out=in_sb[32 * e : 32 * e + KS[e], :], in_=inps[e][:]
                )
            n_copy = 0
            for qidx, (e, v) in enumerate(QUAD_ORDER):
                K = KS[e]
                bp = 32 * e
                ot = ot_pool.tile([128, OTW], F16)
                for bi in range(8):     # i-block of 32 rows
                    W = BIW[bi]
                    ps = ps_pool.tile([128, 512], F32)
                    for c4 in range(4):  # channel within quad
                        sl = v * 4 + c4
                        lo = sl * 256
                        ro = 2048 + sl * 512
                        nc.tensor.matmul(
                            ps[c4 * 32 : (c4 + 1) * 32, 0:W],
                            in_sb[bp : bp + K, lo + bi * 32 : lo + bi * 32 + 32],
                            in_sb[bp : bp + K, ro + 64 * bi : ro + 512],
                            start=True,
                            stop=True,
                            tile_position=(bp, c4 * 32),
                        )
                    dst = ot[:, BIO[bi] : BIO[bi] + W]
                    if n_copy % 2 == 0:
                        nc.scalar.copy(dst, ps[:, 0:W])
                    else:
                        nc.vector.tensor_copy(out=dst, in_=ps[:, 0:W])
                    n_copy += 1
                # alternate stores across the two physical HWDGE rings so
                # SDMA engines can pull from the other ring at store
                # boundaries instead of idling
                eng = nc.sync if qidx % 2 == 0 else nc.scalar
                eng.dma_start(out=out[qidx], in_=ot[:])
    nc.compile()
    return nc


def _host_prep(reps, cid):
    """Build per-core fp16 lhsT/rhs input tensors (one per ell)."""
    b, tq = cid // 4, cid % 4
    im = {}
    for ell in range(NELL):
        rep = reps[ell]
        m = 2 * ell + 1
        s_vec = ((-1.0) ** (ell + np.arange(m))).astype(np.float32)
        arr = np.empty((2 * m, IN_COLS), np.float32)
        for sidx in range(8):
            t = tq * 8 + sidx
            Z = rep[b, :, t]                      # [256, m, 2]
            Zr, Zi = Z[..., 0], Z[..., 1]         # [256, m]
            arr[0:m, sidx * 256 : sidx * 256 + 256] = Zr.T
            arr[m:, sidx * 256 : sidx * 256 + 256] = Zi.T
            FZr = s_vec[:, None] * Zr[:, ::-1].T             # [m, 256]
            FZi = s_vec[:, None] * Zi[:, ::-1].T
            R = np.empty((2 * m, 256, 2), np.float32)
            R[0:m, :, 0] = FZr
            R[m:, :, 0] = -FZi
            R[0:m, :, 1] = FZi
            R[m:, :, 1] = FZr
            ro = 2048 + sidx * 512
            arr[:, ro : ro + 512] = R.reshape(2 * m, 512)
        im[f"inp{ell}"] = arr.astype(np.float16)
    return im


def _run(in_maps, **kw):
    if "nc" not in _NC_CACHE:
        _NC_CACHE["nc"] = _build_bass()
    return run_bass_kernel_spmd(_NC_CACHE["nc"], in_maps, list(range(NCORES)), **kw)


def kernel(rep0, rep1, rep2, rep3, _bass_kw=None):
    reps = [np.ascontiguousarray(np.asarray(r, dtype=np.float32)) for r in (rep0, rep1, rep2, rep3)]
    in_maps = [_host_prep(reps, cid) for cid in range(NCORES)]
    res = _run(in_maps, **(_bass_kw or {}))
    out = np.empty((B, N, N, NELL * TAU, 2), np.float32)
    for cid in range(NCORES):
        b, tq = cid // 4, cid % 4
        arr = res.results[cid]["out"]          # [8, 128, OTW] fp16
        o = np.empty((NELL, 8, 256, 256, 2), np.float32)   # [ell, slot, i, j, c]
        for qidx, (e, v) in enumerate(QUAD_ORDER):
            a = arr[qidx].astype(np.float32)
            for bi in range(8):
                nj = 256 - 32 * bi
                blk = a[:, BIO[bi] : BIO[bi] + BIW[bi]].reshape(4, 32, nj, 2)
                for c4 in range(4):
                    o[e, 4 * v + c4, 32 * bi : 32 * bi + 32, 32 * bi :, :] = blk[c4]
        for bi in range(1, 8):                  # mirror lower block triangle
            r = slice(32 * bi, 32 * bi + 32)
            o[:, :, r, : 32 * bi, :] = o[:, :, : 32 * bi, r, :].transpose(0, 1, 3, 2, 4)
        for e in range(NELL):
            lo = e * TAU + tq * 8
            out[b, :, :, lo : lo + 8, :] = o[e].transpose(1, 2, 0, 3)
    kernel.last_result = res
    return out


# revision 3
# speedup vs baseline: 1.3672x; 1.0225x over previous
"""Trainium2 Bass kernel for nn_DotMatrix.

Math: for each (b, ell, t) the reference computes a complex pairwise dot
matrix O[i,j] = sum_m z[i,m] * w[j,m] where z = rep[b,:,t,:,:] as complex
and w the sign-flipped conjugation partner.  As a real matmul:

  lhsT[k, i]   k = (c,m) stacked: [Zr.T; Zi.T]                 [2m, 256]
  rhs[k, 2j+c'] c'=0: [FZr; -FZi], c'=1: [FZi; FZr]            [2m, 512]
  out = lhsT.T @ rhs  -> [256 i, 512 (j,c)]

with FZr[m',j] = s[m'] * Zr[j, M-1-m'], s[m'] = (-1)^(ell+m').

Precision: bf16 operands with fp32 PSUM accumulation and fp16 stores give
~3e-3 relative error on the final output (gate is 2e-2), so no multi-term
decomposition is needed — the contraction dim stays at K = 2m <= 14, the
input tensors total just 393KB per core, and the PE streams one column
per cycle (fp16 operands would halve that rate; fp16 is only used on the
store side where it halves HBM traffic at no cost).

Symmetry trick: the pairwise matrix is symmetric in (i,j) for both the
real and imaginary components (O[i,j] = O[j,i]), so each channel only
computes 32-row i-blocks against j >= 32*bi — 56.25% of the matrix —
and the host mirrors the lower block-triangle for free.

Sharding: 8 cores = 2 batches x 4 tau-quarters.  Each core owns 32
channels ch = ell*8 + s (t = tq*8 + s).  Four channels (a quad) share
each matmul's 128 PSUM partitions via column tiling (tile_position) —
the four streams run concurrently on disjoint 32-column PE tiles, so a
quad i-block costs one W-column pass.  Each ell lives in its own
32-partition row group (rows 32*ell + [0, 2m)).  The eight i-blocks of a
quad are packed into five single-PSUM-bank tiles — (0), (1,7), (2,6),
(3,5), (4) — so evacuation is five wide copies (f32 -> fp16) with a
fixed ScalarE/VectorE split sized to each engine's measured per-column
rate.  Two quads share each [128, 4608] staging tile so output stores
are 9216B-per-partition-line DMAs (small lines throttle the SDMA
engines), issued on the sync ring with the last pair on the scalar ring.
Host reassembles the full [2,256,256,128,2] output.
"""

import numpy as np
import ml_dtypes

import concourse.bass as bass
import concourse.bacc as bacc
import concourse.mybir as mybir
from concourse.bass_utils import run_bass_kernel_spmd
from concourse.tile import TileContext

B, N, TAU, NELL = 2, 256, 32, 4
NCORES = 8
NCH = 32          # channels per core (4 ell * 8 slots)
F32 = mybir.dt.float32
F16 = mybir.dt.float16
BF16 = mybir.dt.bfloat16
BFNP = ml_dtypes.bfloat16
KS = [2 * (2 * ell + 1) for ell in range(NELL)]       # 2, 6, 10, 14
BIW = [512 - 64 * bi for bi in range(8)]              # cols per 32-row i-block
# PSUM pack: five single-bank tiles per quad; each holds whole i-blocks
PACK = [(0,), (1, 7), (2, 6), (3, 5), (4,)]           # widths 512,512,512,512,256
PACKW = [sum(BIW[b] for b in g) for g in PACK]
# offset of each i-block inside the quad's 2304-col staging span
BIO2 = {}
_off = 0
for _g in PACK:
    for _b in _g:
        BIO2[_b] = _off
        _off += BIW[_b]
OTW = 2304                                            # sum of all widths
QUAD_ORDER = [(0, 0), (0, 1), (3, 0), (3, 1), (2, 0), (2, 1), (1, 0), (1, 1)]
IN_COLS = 8 * 256 + 8 * 512                           # 6144: lhs slots then rhs slots

_NC_CACHE = {}


def _build_bass():
    nc = bacc.Bacc()
    # One input tensor per ell: [2m, 6144] bf16; cols [0:2048) hold the
    # eight 256-wide lhsT slot blocks, [2048:6144) the eight 512-wide rhs
    # slot blocks.  Each lands in SBUF row group 32*ell.
    inps = [
        nc.declare_dram_parameter(f"inp{e}", [KS[e], IN_COLS], BF16, isOutput=False)
        for e in range(NELL)
    ]
    # Output: one row per quad PAIR, 4608 cols (two 2304-col quad spans)
    out = nc.declare_dram_parameter("out", [4, 128, 2 * OTW], F16, isOutput=True)

    with TileContext(nc) as tc:
        with (
            tc.tile_pool(name="lin", bufs=1) as lin_pool,
            tc.tile_pool(name="ps", bufs=8, space="PSUM") as ps_pool,
            tc.tile_pool(name="ot", bufs=3) as ot_pool,
        ):
            in_sb = lin_pool.tile([128, IN_COLS], BF16, name="in_sb")
            # PE pre-warm: dependency-free dummy matmuls keep the PE busy
            # from kernel start so the HAM clock gate is already released
            # when the first real matmuls arrive.
            warm_in = lin_pool.tile([128, 512], BF16, name="warm_in")
            warm_ps = ps_pool.tile([128, 512], F32, tag="ps", name="warm_ps")
            nc.vector.memset(warm_in[:], 0.0)
            for _ in range(4):
                nc.tensor.matmul(
                    warm_ps[:], warm_in[:, 0:128], warm_in[:, 0:512],
                    start=True, stop=True,
                )
            # Input loads ride the sync HWDGE ring, in the order quads
            # consume them (ell0 first).
            for e in (0, 3, 2, 1):
                nc.sync.dma_start(
                    out=in_sb[32 * e : 32 * e + KS[e], :], in_=inps[e][:]
                )
            ot = None
            for qidx, (e, v) in enumerate(QUAD_ORDER):
                K = KS[e]
                bp = 32 * e
                if qidx % 2 == 0:
                    ot = ot_pool.tile([128, 2 * OTW], F16)
                qoff = (qidx % 2) * OTW
                for gi, grp in enumerate(PACK):
                    ps = ps_pool.tile([128, 512], F32)
                    poff = 0
                    for bi in grp:
                        W = BIW[bi]
                        for c4 in range(4):  # channel within quad
                            sl = v * 4 + c4
                            lo = sl * 256
                            ro = 2048 + sl * 512
                            nc.tensor.matmul(
                                ps[c4 * 32 : (c4 + 1) * 32, poff : poff + W],
                                in_sb[bp : bp + K, lo + bi * 32 : lo + bi * 32 + 32],
                                in_sb[bp : bp + K, ro + 64 * bi : ro + 512],
                                start=True,
                                stop=True,
                                tile_position=(bp, c4 * 32),
                            )
                        poff += W
                    dst = ot[:, qoff + BIO2[grp[0]] : qoff + BIO2[grp[0]] + PACKW[gi]]
                    # scalar is the faster PSUM reader (0.84 ns/col vs 1.28):
                    # give it three of the five tiles (1280 of 2304 cols)
                    if gi in (0, 2, 4):
                        nc.scalar.copy(dst, ps[:, 0 : PACKW[gi]])
                    else:
                        nc.vector.tensor_copy(out=dst, in_=ps[:, 0 : PACKW[gi]])
                if qidx % 2 == 1:
                    # 2-quad store: 9216B per partition line keeps the SDMA
                    # engines at full packet efficiency.  Last pair goes on
                    # the scalar ring (its copies are done by then) so two
                    # transfers can drain concurrently at the end.
                    eng = nc.scalar if qidx == 7 else nc.sync
                    eng.dma_start(out=out[qidx // 2], in_=ot[:])
    nc.compile()
    return nc


def _host_prep(reps, cid):
    """Build per-core bf16 lhsT/rhs input tensors (one per ell)."""
    b, tq = cid // 4, cid % 4
    im = {}
    for ell in range(NELL):
        rep = reps[ell]
        m = 2 * ell + 1
        s_vec = ((-1.0) ** (ell + np.arange(m))).astype(np.float32)
        arr = np.empty((2 * m, IN_COLS), np.float32)
        for sidx in range(8):
            t = tq * 8 + sidx
            Z = rep[b, :, t]                      # [256, m, 2]
            Zr, Zi = Z[..., 0], Z[..., 1]         # [256, m]
            arr[0:m, sidx * 256 : sidx * 256 + 256] = Zr.T
            arr[m:, sidx * 256 : sidx * 256 + 256] = Zi.T
            FZr = s_vec[:, None] * Zr[:, ::-1].T             # [m, 256]
            FZi = s_vec[:, None] * Zi[:, ::-1].T
            R = np.empty((2 * m, 256, 2), np.float32)
            R[0:m, :, 0] = FZr
            R[m:, :, 0] = -FZi
            R[0:m, :, 1] = FZi
            R[m:, :, 1] = FZr
            ro = 2048 + sidx * 512
            arr[:, ro : ro + 512] = R.reshape(2 * m, 512)
        im[f"inp{ell}"] = arr.astype(BFNP)
    return im


def _run(in_maps, **kw):
    if "nc" not in _NC_CACHE:
        _NC_CACHE["nc"] = _build_bass()
    return run_bass_kernel_spmd(_NC_CACHE["nc"], in_maps, list(range(NCORES)), **kw)


def kernel(rep0, rep1, rep2, rep3, _bass_kw=None):
    reps = [np.ascontiguousarray(np.asarray(r, dtype=np.float32)) for r in (rep0, rep1, rep2, rep3)]
    in_maps = [_host_prep(reps, cid) for cid in range(NCORES)]
    res = _run(in_maps, **(_bass_kw or {}))
    out = np.empty((B, N, N, NELL * TAU, 2), np.float32)
    for cid in range(NCORES):
        b, tq = cid // 4, cid % 4
        arr = res.results[cid]["out"]          # [4, 128, 4608] fp16
        o = np.empty((NELL, 8, 256, 256, 2), np.float32)   # [ell, slot, i, j, c]
        for qidx, (e, v) in enumerate(QUAD_ORDER):
            a = arr[qidx // 2, :, (qidx % 2) * OTW : (qidx % 2) * OTW + OTW].astype(
                np.float32
            )
            for bi in range(8):
                nj = 256 - 32 * bi
                blk = a[:, BIO2[bi] : BIO2[bi] + BIW[bi]].reshape(4, 32, nj, 2)
                for c4 in range(4):
                    o[e, 4 * v + c4, 32 * bi : 32 * bi + 32, 32 * bi :, :] = blk[c4]
        for bi in range(1, 8):                  # mirror lower block triangle
            r = slice(32 * bi, 32 * bi + 32)
            o[:, :, r, : 32 * bi, :] = o[:, :, : 32 * bi, r, :].transpose(0, 1, 3, 2, 4)
        for e in range(NELL):
            lo = e * TAU + tq * 8
            out[b, :, :, lo : lo + 8, :] = o[e].transpose(1, 2, 0, 3)
    kernel.last_result = res
    return out
